# revision 16
# baseline (speedup 1.0000x reference)
"""Distributed Trainium2 kernel for ArticulatoryMetricLoss.

loss = mean_{i != j} ((||e_i||^2 + ||e_j||^2 - 2 e_i.e_j) - art_dist[i, j])^2

Strategy (8 NeuronCores), v3 — uniform fp8 DoubleRow PE stream:
  - Host symmetrizes art: a~ = (A + A^T)/2; per ordered pair
      (d2 - a_ij)^2 + (d2 - a_ji)^2 = 2 (d2 - a~_ij)^2 + (a_ij - a_ji)^2 / 2,
    so the device sums (d2 - a~)^2 over the 36-unit symmetric block cover
    (off-diag units weight 2, diag units weight 1); the antisymmetric part
    is an exact host term.
  - E in fp8 e4m3 (IEEE, max 240): 3 DoubleRow matmuls per sub-job contract
    K=256 each (2 k-tiles per instruction, the real source of fp8's 2x).
  - art tiles ship as art_h = (-a~ + s~_i + s~_j - K)/2 in e4m3 (range fits
    240), stochastically rounded (RTN on this bf16-grid-structured data has
    a systematic bias). K = 1568.
  - Per sub-job, one of two drain paths (keeps PE/DVE/ACT balanced):
      inject: a 4th DR matmul with stationary [2*I; 0] adds 2*art_h into
        PSUM (PSUM = w - K, w = d2 - a~); ACT squares via
        Square(psum*1 + K), or DVE copies bf16 u and accumulates
        (u + 2K)*u = w^2 - K^2.
      subtract: no 4th matmul; DVE stt u' = 0.5*psum - art_h (= (K - w)/2);
        ACT squares via Square(-2*u' + K), or DVE accumulates
        (u' - K)*u' = (w^2 - K^2)/4.
    All PE instructions are fp8-e4m3 DoubleRow: mixing a plain bf16 matmul
    into a DR accumulation group mis-pairs the stationary rows on HW
    (verified by probe).
  - Host combine: weighted A1 sums (+512K^2 / x4 fixups for DVE columns) +
    exact corrections: C (antisym art), D (device diag terms), T1 (exact
    first-order norm-quantization correction). Validated ~1e-6 rel err.
"""

import os
import sys
from contextlib import ExitStack

import numpy as np

for _p in ("/opt/trn_rl_repo", "/root/.axon_site/_ro/trn_rl_repo"):
    if os.path.isdir(_p) and _p not in sys.path:
        sys.path.insert(0, _p)

import ml_dtypes

import concourse.tile as tile
from concourse import bacc, mybir
from concourse.bass_utils import run_bass_kernel_spmd

B = 4096          # rows/cols of the pairwise matrix
D = 768           # embedding dim
NCORES = 8
BLK = 512         # i block size (8x8 block grid)
P = 128           # SBUF partitions
KP = 3            # DoubleRow k-pairs (6 k-tiles of 128 -> 3 pairs of 256)
NSUB = 18         # sub-jobs per core
PAIRS = B * (B - 1)
KC = 1568.0       # centering constant for art tiles

BF16 = mybir.dt.bfloat16
F32 = mybir.dt.float32
F8 = mybir.dt.float8e4

# drain-path assignment per sub-job (PE 13.0 / DVE 12.4 / ACT 12.2 us):
INJ_ACT = (1, 4, 8, 13)       # 4th DR matmul injects art; ACT squares
INJ_DVE = (10, 17)            # inject; DVE copies + squares
SUB_ACT = (0, 2, 3, 5, 6, 9, 11, 12, 15, 16)  # DVE stt subtract; ACT squares
SUB_DVE = (7, 14)             # DVE stt subtract; DVE squares
INJ = set(INJ_ACT) | set(INJ_DVE)
DVE_SQ = set(INJ_DVE) | set(SUB_DVE)
N_WARM = 10

SUB_SLOT = [0] * 12 + [1] * 2 + [0] * 4


def subjobs(c):
    """(bi, bj, jt, weight) per sub-job; bi = moving block, (bj, jt) = the
    128-row stationary j-tile. lhs slot 0 = block c, slot 1 = block c%4."""
    jobs = []
    for d in (1, 2, 3):
        for jt in range(4):
            jobs.append((c, (c + d) % 8, jt, 2))
    p = c % 4
    for q in range(2):
        jt = q if c < 4 else q + 2
        jobs.append((p, p + 4, jt, 2))
    for jt in range(4):
        jobs.append((c, c, jt, 1))
    return jobs


def build_graph():
    nc = bacc.Bacc("TRN2", target_bir_lowering=False, debug=False, num_devices=NCORES)

    LHS_W = 2 * KP * 2 * BLK        # 2 slots x 3 kpairs x 2 planes x 512
    RHS_W = NSUB * KP * 2 * P       # 18 subjobs x 3 kpairs x 2 planes x 128
    ART_W = NSUB * BLK              # 18 subjobs x 512 (fp8, half-scaled)

    lhs_d = nc.dram_tensor("lhs", [P, LHS_W], F8, kind="ExternalInput")
    rhs_d = nc.dram_tensor("rhs", [P, RHS_W], F8, kind="ExternalInput")
    art_d = nc.dram_tensor("art", [P, ART_W], F8, kind="ExternalInput")
    idn_d = nc.dram_tensor("ident", [P, 2 * P], F8, kind="ExternalInput")
    a1a_d = nc.dram_tensor("a1a", [P, 14], F32, kind="ExternalOutput")
    a1d_d = nc.dram_tensor("a1d", [P, 4], F32, kind="ExternalOutput")

    with tile.TileContext(nc) as tc, ExitStack() as ctx:
        sb_pool = ctx.enter_context(tc.tile_pool(name="sb", bufs=1))
        scr_pool = ctx.enter_context(tc.tile_pool(name="scr", bufs=4))
        psum_pool = ctx.enter_context(tc.tile_pool(name="psum", bufs=7, space="PSUM"))
        psw_pool = ctx.enter_context(tc.tile_pool(name="psw", bufs=1, space="PSUM"))
        u_pool = scr_pool
        const_pool = acc_pool = sb_pool

        lhs_t = sb_pool.tile([P, LHS_W], F8, name="lhs")
        rhs_t = sb_pool.tile([P, RHS_W], F8, name="rhs")
        art_t = sb_pool.tile([P, (NSUB + 1) * BLK], F8, name="art")
        idn = const_pool.tile([P, 2 * P], F8, name="idn")
        warm_st = const_pool.tile([P, 2 * P], F8, name="warmst")
        warm_mv = const_pool.tile([P, 2 * P], F8, name="warmmv")

        # ---- warmup operands + art pad: tiny memsets, no DMA deps.
        nc.vector.memset(warm_st[:], 1.0)
        nc.vector.memset(warm_mv[:], 0.5)
        nc.vector.memset(art_t[:, NSUB * BLK :], 0.0)
        kc_bias = const_pool.tile([P, 1], F32, name="kcbias")
        nc.vector.memset(kc_bias[:], KC)

        # ---- DMA: first-needed transfers first, spread across queues.
        def load_rhs(t0, t1, eng):
            eng.dma_start(
                rhs_t[:, t0 * KP * 2 * P : t1 * KP * 2 * P],
                rhs_d[:, t0 * KP * 2 * P : t1 * KP * 2 * P],
            )

        def load_lhs(L, kk, eng):
            off = (L * KP + kk) * 2 * BLK
            eng.dma_start(lhs_t[:, off : off + 2 * BLK], lhs_d[:, off : off + 2 * BLK])

        def load_art(t0, t1, eng):
            eng.dma_start(art_t[:, t0 * BLK : t1 * BLK], art_d[:, t0 * BLK : t1 * BLK])

        # lhs (both slots, one fat transfer) + back-half rhs on sync;
        # front rhs + ident + front art on scalar; back art on gpsimd.
        nc.sync.dma_start(lhs_t[:], lhs_d[:])
        load_rhs(6, 18, nc.sync)

        load_rhs(0, 6, nc.scalar)
        nc.scalar.dma_start(idn[:], idn_d[:])
        load_art(0, 6, nc.scalar)

        load_art(6, 12, nc.gpsimd)
        load_art(12, 18, nc.gpsimd)

        def rhs_view(t, kk):  # stationary [128, 2, 128] fp8
            off = (t * KP + kk) * 2 * P
            return rhs_t[:, off : off + 2 * P].rearrange("p (two f) -> p two f", two=2)

        def lhs_view(L, kk):  # moving [128, 2, 512] fp8
            off = (L * KP + kk) * 2 * BLK
            return lhs_t[:, off : off + 2 * BLK].rearrange(
                "p (two f) -> p two f", two=2
            )

        def art_view(t):      # [128, 512] fp8 (j = partition), half-scaled
            return art_t[:, t * BLK : (t + 1) * BLK]

        def art_inj_view(t):  # [128, 2, 512]: plane 1 hits the 0-stationary
            return art_t[:, t * BLK : (t + 2) * BLK].rearrange(
                "p (two f) -> p two f", two=2
            )

        # ---- PE p-state warmup: small DR matmuls while DMA fills.
        warm_ps = psw_pool.tile([P, P], F32, name="warm_ps")
        for w in range(N_WARM):
            nc.tensor.matmul(
                warm_ps[:],
                warm_st[:].rearrange("p (two f) -> p two f", two=2),
                warm_mv[:].rearrange("p (two f) -> p two f", two=2),
                start=True,
                stop=True,
                perf_mode=mybir.MatmulPerfMode.DoubleRow,
            )

        # ---- main loop: 18 sub-jobs.
        A1a = acc_pool.tile([P, 14], F32, name="A1a")
        A1d = acc_pool.tile([P, 4], F32, name="A1d")
        acol = {}
        dcol = {}
        for t in range(NSUB):
            if t in DVE_SQ:
                dcol[t] = len(dcol)
            else:
                acol[t] = len(acol)

        for t in range(NSUB):
            ps = psum_pool.tile([P, BLK], F32, tag="ps", name=f"ps{t}")
            L = SUB_SLOT[t]
            for kk in range(KP):
                nc.tensor.matmul(
                    ps[:],
                    rhs_view(t, kk),
                    lhs_view(L, kk),
                    start=(kk == 0),
                    stop=(kk == KP - 1 and t not in INJ),
                    perf_mode=mybir.MatmulPerfMode.DoubleRow,
                )
            if t in INJ:
                # psum += 2 * art_h  (stationary = [2I; 0])
                nc.tensor.matmul(
                    ps[:],
                    idn[:].rearrange("p (two f) -> p two f", two=2),
                    art_inj_view(t),
                    start=False,
                    stop=True,
                    perf_mode=mybir.MatmulPerfMode.DoubleRow,
                )
                so = scr_pool.tile([P, BLK], BF16, tag="scr", name=f"so{t}")
                if t in DVE_SQ:   # u = w - K; (u + 2K) u = w^2 - K^2
                    u = u_pool.tile([P, BLK], BF16, tag="u", name=f"u{t}")
                    nc.vector.tensor_copy(u[:], ps[:])
                    nc.vector.scalar_tensor_tensor(
                        out=so[:],
                        in0=u[:],
                        scalar=2.0 * KC,
                        in1=u[:],
                        op0=mybir.AluOpType.add,
                        op1=mybir.AluOpType.mult,
                        accum_out=A1d[:, dcol[t] : dcol[t] + 1],
                    )
                else:             # (psum + K)^2 = w^2
                    nc.scalar.activation(
                        so[:],
                        ps[:],
                        mybir.ActivationFunctionType.Square,
                        bias=kc_bias[:],
                        accum_out=A1a[:, acol[t] : acol[t] + 1],
                    )
            else:
                # u' = 0.5 psum - art_h = (K - w)/2
                u = u_pool.tile([P, BLK], BF16, tag="u", name=f"u{t}")
                nc.vector.scalar_tensor_tensor(
                    out=u[:],
                    in0=ps[:],
                    scalar=0.5,
                    in1=art_view(t),
                    op0=mybir.AluOpType.mult,
                    op1=mybir.AluOpType.subtract,
                )
                so = scr_pool.tile([P, BLK], BF16, tag="scr", name=f"so{t}")
                if t in DVE_SQ:   # (u' - K) u' = (w^2 - K^2)/4
                    nc.vector.scalar_tensor_tensor(
                        out=so[:],
                        in0=u[:],
                        scalar=-KC,
                        in1=u[:],
                        op0=mybir.AluOpType.add,
                        op1=mybir.AluOpType.mult,
                        accum_out=A1d[:, dcol[t] : dcol[t] + 1],
                    )
                else:             # (-2 u' + K)^2 = w^2
                    nc.scalar.activation(
                        so[:],
                        u[:],
                        mybir.ActivationFunctionType.Square,
                        bias=kc_bias[:],
                        scale=-2.0,
                        accum_out=A1a[:, acol[t] : acol[t] + 1],
                    )

        nc.sync.dma_start(a1a_d[:], A1a[:])
        nc.sync.dma_start(a1d_d[:], A1d[:])

    nc.compile()
    return nc


_CACHED = {}


def _sr_fp8(x32, seed):
    """Exact stochastic rounding f32 -> IEEE e4m3 (unbiased), f32 on-grid."""
    f8 = ml_dtypes.float8_e4m3
    rng = np.random.default_rng(seed)
    x = np.ascontiguousarray(x32, dtype=np.float32)
    q = x.astype(f8)
    qf = q.astype(np.float32)
    qb = q.view(np.uint8).astype(np.int16)
    need_up = np.abs(x) > np.abs(qf)
    nb = qb + np.where(need_up, 1, -1)
    zero_mask = (qf == 0.0) & ~need_up & (x != 0.0)
    nb = np.where(zero_mask, np.where(x < 0, 0x81, 0x01), nb)
    nb = np.clip(nb, 0, 255).astype(np.uint8)
    nf = nb.view(f8).astype(np.float32)
    nf = np.where(np.isfinite(nf), nf, qf)
    lo = np.minimum(qf, nf)
    hi = np.maximum(qf, nf)
    denom = np.where(hi > lo, hi - lo, 1.0)
    p_hi = np.clip((x - lo) / denom, 0.0, 1.0)
    u = rng.random(x.shape, dtype=np.float32)
    y = np.where(x == qf, qf, np.where(u < p_hi, hi, lo))
    return y.astype(np.float32)


def shard_inputs(embeddings: np.ndarray, art_dist: np.ndarray):
    bf16 = ml_dtypes.bfloat16
    f8 = ml_dtypes.float8_e4m3
    E = np.ascontiguousarray(embeddings, dtype=np.float32)
    A = np.ascontiguousarray(art_dist, dtype=np.float32)

    E8 = E.astype(f8).astype(np.float32)
    s_q32 = np.einsum("id,id->i", E8, E8, dtype=np.float32)
    s_bf = s_q32.astype(bf16).astype(np.float32)
    At = (A + A.T) * np.float32(0.5)
    E8T = np.ascontiguousarray(E8.T)            # [D, B]
    M2T = np.ascontiguousarray((-2.0 * E8).T)   # [D, B]
    idn = np.zeros((P, 2 * P), np.float32)
    idn[:, :P] = 2.0 * np.eye(P, dtype=np.float32)
    idn = idn.astype(f8)

    def pack_pairs(MT, cols):
        """[D, B] slab -> [128, KP*2*width] k-pair plane-major packing."""
        sl = MT[:, cols]
        w = sl.shape[1]
        return np.ascontiguousarray(
            sl.reshape(KP, 2, P, w).transpose(2, 0, 1, 3).reshape(P, KP * 2 * w)
        )

    diag_dev = np.zeros(B, np.float64)
    s_q64 = np.einsum(
        "id,id->i", E8.astype(np.float64), E8.astype(np.float64)
    )
    in_maps = []
    for c in range(NCORES):
        jobs = subjobs(c)
        lhs = np.concatenate(
            [
                pack_pairs(M2T, slice(b * BLK, (b + 1) * BLK)).astype(f8)
                for b in (c, c % 4)
            ],
            axis=1,
        )
        rhs = np.concatenate(
            [
                pack_pairs(
                    E8T, slice(bj * BLK + jt * P, bj * BLK + (jt + 1) * P)
                ).astype(f8)
                for (bi, bj, jt, w) in jobs
            ],
            axis=1,
        )
        tiles = []
        for t, (bi, bj, jt, w) in enumerate(jobs):
            J = slice(bj * BLK + jt * P, bj * BLK + (jt + 1) * P)
            I = slice(bi * BLK, (bi + 1) * BLK)
            raw_h = (
                (-At[J, I] + s_bf[None, I] + s_bf[J, None] - np.float32(KC))
                * np.float32(0.5)
            ).astype(np.float32)
            tq = _sr_fp8(raw_h, seed=(c * 64 + t) * 7919 + 13)
            if bi == bj:  # diagonal unit: record device diag terms exactly
                jj = np.arange(jt * P, (jt + 1) * P)
                gi = bi * BLK + jj
                w_ii = (
                    -2.0 * s_q64[gi]
                    + 2.0 * tq[np.arange(P), jj].astype(np.float64)
                    + KC
                )
                diag_dev[gi] = np.square(w_ii)
            tiles.append(tq.astype(f8))
        art = np.concatenate(tiles, axis=1)
        in_maps.append(
            {
                "lhs": np.ascontiguousarray(lhs),
                "rhs": np.ascontiguousarray(rhs),
                "art": np.ascontiguousarray(art),
                "ident": idn,
            }
        )

    # ---- host-exact corrections
    Ad = A.astype(np.float64)
    C_host = 0.25 * np.square(Ad - Ad.T).sum()
    D_diag = diag_dev.sum()

    E64 = E.astype(np.float64)
    E8_64 = E8.astype(np.float64)
    s_t = np.einsum("id,id->i", E64, E64)
    sb = s_bf.astype(np.float64)
    Di = sb - s_t
    t8 = E8_64.sum(0)
    tt = E64.sum(0)
    row_d2q_off = (B * sb + sb.sum() - 2.0 * (E8_64 @ t8)) - (2.0 * sb - 2.0 * s_q64)
    row_d2t_off = B * s_t + s_t.sum() - 2.0 * (E64 @ tt)
    row_a_off = (Ad + Ad.T).sum(1) - 2.0 * np.diag(Ad)
    R = 2.0 * (row_d2q_off + row_d2t_off) - 2.0 * row_a_off
    T1 = (Di * R).sum()

    host = {"C_host": C_host, "D_diag": D_diag, "T1": T1}
    return in_maps, host


def combine(results, host):
    """Host unshard: weighted A1 sums + exact corrections."""
    acol = {}
    dcol = {}
    for t in range(NSUB):
        if t in DVE_SQ:
            dcol[t] = len(dcol)
        else:
            acol[t] = len(acol)
    K2 = np.float64(KC) * KC
    T = 0.0
    for c in range(NCORES):
        jobs = subjobs(c)
        A1a = results[c]["a1a"].astype(np.float64)
        A1d = results[c]["a1d"].astype(np.float64)
        for t, (bi, bj, jt, w) in enumerate(jobs):
            if t in DVE_SQ:
                col = A1d[:, dcol[t]]
                if t in INJ:      # (w+K)(w-K) accumulated
                    val = col + BLK * K2
                else:             # (u'-K)u' accumulated, u' = (K-w)/2
                    val = 4.0 * col + BLK * K2
            else:
                val = A1a[:, acol[t]]
            T += w * val.sum()
    loss = (T - host["D_diag"] + host["C_host"] - host["T1"]) / PAIRS
    return np.float32(loss)


def _get_nc():
    if "nc" not in _CACHED:
        _CACHED["nc"] = build_graph()
    return _CACHED["nc"]


def _ensure_ntff_hook():
    """The agent image's antenv package lacks axon_hooks, so trace=True in
    run_bass_kernel_spmd crashes on import. Recreate the module + register
    the ctypes NTFF hook the way trn_boot would have."""
    try:
        from antenv.axon_hooks import get_axon_ntff_profile_hook  # noqa: F401

        return
    except ImportError:
        pass
    import types

    import antenv

    mod = types.ModuleType("antenv.axon_hooks")
    holder = {"hook": None}
    mod.set_axon_ntff_profile_hook = lambda h: holder.__setitem__("hook", h)
    mod.get_axon_ntff_profile_hook = lambda: holder["hook"]
    sys.modules["antenv.axon_hooks"] = mod
    antenv.axon_hooks = mod
    try:
        from trn_agent_boot.trn_boot import _ntff_profile_via_ctypes

        for so in ("/opt/axon/libaxon_pjrt.so",):
            if os.path.exists(so):
                holder["hook"] = _ntff_profile_via_ctypes(so)
                break
    except Exception as e:  # degrade: tracing skipped, run still works
        print(f"ntff hook setup failed ({e}); tracing disabled", file=sys.stderr)


def run(embeddings: np.ndarray, art_dist: np.ndarray, **run_kwargs):
    if run_kwargs.get("trace"):
        _ensure_ntff_hook()
    nc = _get_nc()
    in_maps, host = shard_inputs(np.asarray(embeddings), np.asarray(art_dist))
    res = run_bass_kernel_spmd(nc, in_maps, core_ids=list(range(NCORES)), **run_kwargs)
    loss = combine(res.results, host)
    return np.asarray(loss, dtype=np.float32), res


def kernel(embeddings: np.ndarray, art_dist: np.ndarray) -> np.ndarray:
    loss, _ = run(embeddings, art_dist)
    return loss
